# revision 1
# baseline (speedup 1.0000x reference)
"""Multi-head attention (B=4, S=2048, D=1024, H=16, HS=64, causal) on 8 trn2 cores.

Strategy: tensor-parallel over heads (2 heads per core), x replicated.
Per core: QKV projections (fp32r matmuls), causal attention with
transposed-scores softmax (no max-subtraction; scores are O(1) by
construction), output projection of the local head pair -> partial
[B*S, D]. Host sums the 8 partials (the Wo contraction over heads).

All matmuls run in float32r (fp32 with 12-bit-significand rounding,
bit-compatible with fp32). Moving free dim is kept at 256: measured on
HW, fp32r streams 1 cycle/row at N<=256 (~148 ns/matmul) but ~2
cycles/row at N=512.
"""

import sys

sys.path.insert(0, "/opt/trn_rl_repo")

import numpy as np

import concourse.bacc as bacc
import concourse.bass as bass
import concourse.mybir as mybir
import concourse.tile as tile
from concourse.bass_utils import run_bass_kernel_spmd

F32 = mybir.dt.float32
F32R = mybir.dt.float32r
EXP = mybir.ActivationFunctionType.Exp
MUL = mybir.AluOpType.mult

B, S, D, H, HS = 4, 2048, 1024, 16, 64
ROWS = B * S                      # 8192
NB = 8                            # cores
SCALE = 1.0 / float(np.sqrt(HS))  # 0.125
CH = 256                          # matmul moving width (fp32r fast path)
NC_CH = S // CH                   # 8 q/row chunks per batch
NKT = S // 128                    # 16 k-tiles per batch
NQT = S // 128                    # 16 row-tiles per batch

# knobs for test.py
ABLATE = set()   # subset of {"attn", "outproj", "exp_on_dve"}
TRACE = False
LAST_RESULTS = None
LAST_IN_MAPS = None


def round_fp32r(a: np.ndarray) -> np.ndarray:
    """Round fp32 to fp32r (12-bit significand, round-to-nearest-even)."""
    u = np.ascontiguousarray(a, dtype=np.float32).view(np.uint32)
    low = u & np.uint32(0xFFF)
    hi = u >> np.uint32(12)
    rnd = (low > 0x800) | ((low == 0x800) & ((hi & 1) == 1))
    return ((hi + rnd.astype(np.uint32)) << np.uint32(12)).view(np.float32)


def _build_nc(reps: int = 1):
    nc = bacc.Bacc()

    xT = nc.declare_dram_parameter("xT", [D, ROWS], F32R, isOutput=False)
    wq = nc.declare_dram_parameter("wq", [D, 128], F32R, isOutput=False)
    wk = nc.declare_dram_parameter("wk", [D, 128], F32R, isOutput=False)
    wv = nc.declare_dram_parameter("wv", [D, 128], F32R, isOutput=False)
    wo0 = nc.declare_dram_parameter("wo0", [64, D], F32R, isOutput=False)
    wo1 = nc.declare_dram_parameter("wo1", [64, D], F32R, isOutput=False)
    bq = nc.declare_dram_parameter("bq", [128, 1], F32, isOutput=False)
    bk = nc.declare_dram_parameter("bk", [128, 1], F32, isOutput=False)
    bv = nc.declare_dram_parameter("bv", [128, 1], F32, isOutput=False)
    masks = nc.declare_dram_parameter("masks", [128, 2 * CH], F32R, isOutput=False)
    ident = nc.declare_dram_parameter("ident", [128, 128], F32R, isOutput=False)
    ones = nc.declare_dram_parameter("ones", [128, 128], F32R, isOutput=False)
    out = nc.declare_dram_parameter("out", [ROWS, D], F32, isOutput=True)

    with tile.TileContext(nc) as tc:
        with tc.tile_pool(name="const", bufs=1) as cpool, \
             tc.tile_pool(name="sb", bufs=1) as sb, \
             tc.tile_pool(name="ps", bufs=1, space="PSUM") as ps:

            # ---- persistent constants ----
            wq_sb = cpool.tile([128, 8 * 128], F32R)
            wk_sb = cpool.tile([128, 8 * 128], F32R)
            wv_sb = cpool.tile([128, 8 * 128], F32R)
            for w_sb, w in ((wq_sb, wq), (wk_sb, wk), (wv_sb, wv)):
                nc.sync.dma_start(
                    w_sb[:, :].rearrange("p (dk j) -> p dk j", dk=8),
                    w[:, :].rearrange("(dk p) j -> p dk j", dk=8),
                )
            wo0_sb = cpool.tile([64, D], F32R)
            wo1_sb = cpool.tile([64, D], F32R)
            nc.sync.dma_start(wo0_sb[:, :], wo0[:, :])
            nc.sync.dma_start(wo1_sb[:, :], wo1[:, :])
            bq_sb = cpool.tile([128, 1], F32)
            bk_sb = cpool.tile([128, 1], F32)
            bv_sb = cpool.tile([128, 1], F32)
            nc.sync.dma_start(bq_sb[:, :], bq[:, :])
            nc.sync.dma_start(bk_sb[:, :], bk[:, :])
            nc.sync.dma_start(bv_sb[:, :], bv[:, :])
            masks_sb = cpool.tile([128, 2 * CH], F32R)
            nc.sync.dma_start(masks_sb[:, :], masks[:, :])
            ident_sb = cpool.tile([128, 128], F32R)
            nc.sync.dma_start(ident_sb[:, :], ident[:, :])
            ones_sb = cpool.tile([128, 128], F32R)
            nc.sync.dma_start(ones_sb[:, :], ones[:, :])

            env = dict(
                wq_sb=wq_sb, wk_sb=wk_sb, wv_sb=wv_sb, wo0_sb=wo0_sb,
                wo1_sb=wo1_sb, bq_sb=bq_sb, bk_sb=bk_sb, bv_sb=bv_sb,
                masks_sb=masks_sb, ident_sb=ident_sb, ones_sb=ones_sb,
                xT=xT, out=out)

            if reps > 1:
                with tc.For_i(0, reps, 1):
                    _emit(nc, tc, sb, ps, env)
            else:
                _emit(nc, tc, sb, ps, env)
    nc.compile()
    return nc


def _emit(nc, tc, sb, ps, env):
    wq_sb = env["wq_sb"]; wk_sb = env["wk_sb"]; wv_sb = env["wv_sb"]
    wo0_sb = env["wo0_sb"]; wo1_sb = env["wo1_sb"]
    bq_sb = env["bq_sb"]; bk_sb = env["bk_sb"]; bv_sb = env["bv_sb"]
    masks_sb = env["masks_sb"]; ident_sb = env["ident_sb"]
    ones_sb = env["ones_sb"]; xT = env["xT"]; out = env["out"]

    st_ = {}   # per-b tiles

    def qkv_unit(b, cp):
        def run():
            r0 = b * S
            if cp == 0:
                st_["qT", b] = sb.tile([128, S], F32R, tag="qT", bufs=2,
                                       name=f"qT{b}")
                st_["kT", b] = sb.tile([128, S], F32R, tag="kT", bufs=2,
                                       name=f"kT{b}")
                st_["vT", b] = sb.tile([128, S], F32R, tag="vT", bufs=2,
                                       name=f"vT{b}")
            qT, kT, vT = st_["qT", b], st_["kT", b], st_["vT", b]
            xt = sb.tile([128, 8 * 2 * CH], F32R, tag="xt", bufs=2,
                         name=f"xt{b}_{cp}")
            nc.sync.dma_start(
                xt[:, :].rearrange("p (dk j) -> p dk j", dk=8),
                xT[:, r0 + cp * 2 * CH: r0 + (cp + 1) * 2 * CH]
                .rearrange("(dk p) j -> p dk j", dk=8),
            )
            for w_sb, dest, bias in ((wq_sb, qT, bq_sb),
                                     (wk_sb, kT, bk_sb),
                                     (wv_sb, vT, bv_sb)):
                pp = ps.tile([128, 512], F32, tag="st", bufs=2,
                             name=f"pp{b}{cp}")
                for half in range(2):
                    for dk in range(8):
                        nc.tensor.matmul(
                            pp[:, half * CH:(half + 1) * CH],
                            w_sb[:, dk * 128:(dk + 1) * 128],
                            xt[:, dk * 2 * CH + half * CH:
                               dk * 2 * CH + (half + 1) * CH],
                            start=(dk == 0), stop=(dk == 7),
                        )
                nc.vector.tensor_scalar_add(
                    dest[:, cp * 2 * CH:(cp + 1) * 2 * CH], pp[:, :],
                    bias[:, :])
        return run

    def v1_unit(b, h):
        def run():
            vT = st_["vT", b]
            v1 = sb.tile([128, NKT * 65], F32R, tag="v1", bufs=4,
                         name=f"v1_{b}_{h}")
            st_["v1", b, h] = v1
            ones_dst = bass.AP(
                v1.tensor, v1.offset + 64,
                [v1.ap[0]] + [[65, NKT]])
            nc.vector.tensor_copy(ones_dst, ones_sb[:, 0:NKT])
            for rt in range(NKT):
                pv = ps.tile([128, 64], F32R, tag="bcx", bufs=2,
                             name=f"pv{b}{h}{rt}")
                nc.tensor.transpose(
                    pv[:, :],
                    vT[h * 64:(h + 1) * 64, rt * 128:(rt + 1) * 128],
                    ident_sb[h * 64:(h + 1) * 64, h * 64:(h + 1) * 64],
                )
                nc.vector.tensor_copy(
                    v1[:, rt * 65: rt * 65 + 64], pv[:, :])
        return run

    def attn_unit(b, c, h):
        def run():
            if ("oT", b) not in st_:
                st_["oT", b] = sb.tile([64, 2 * S], F32R, tag="oT", bufs=2,
                                       name=f"oT{b}")
                st_["den", b] = sb.tile([128, 2 * S], F32R, tag="den", bufs=2,
                                        name=f"den{b}")
            oT, den_r = st_["oT", b], st_["den", b]
            qT, kT = st_["qT", b], st_["kT", b]
            v1 = st_["v1", b, h]
            hp = h * 64
            qs = c * CH
            nk = 2 * (c + 1)
            po = ps.tile([128, CH], F32, tag="po", bufs=2, name=f"po{b}{h}{c}")
            for g in range((nk + 3) // 4):
                k0 = 4 * g
                gn = min(4, nk - k0)
                stp = ps.tile([128, 1024], F32, tag="st", bufs=2,
                              name=f"stp{b}{h}{c}{g}")
                for j in range(gn):
                    nc.tensor.matmul(
                        stp[:, j * CH:(j + 1) * CH],
                        kT[hp:hp + 64, (k0 + j) * 128:(k0 + j + 1) * 128],
                        qT[hp:hp + 64, qs:qs + CH],
                        start=True, stop=True,
                    )
                pt = sb.tile([128, 1024], F32R, tag="pt", bufs=4,
                             name=f"pt{b}{h}{c}{g}")
                if "no_exp" in ABLATE:
                    nc.vector.tensor_copy(pt[0:1, 0:gn * CH],
                                          stp[0:1, 0:gn * CH])
                else:
                    nc.scalar.activation(pt[:, 0:gn * CH], stp[:, 0:gn * CH],
                                         EXP, scale=SCALE)
                    for j in range(gn):
                        m = (k0 + j) - 2 * c
                        if m >= 0:
                            nc.vector.tensor_tensor(
                                pt[:, j * CH:(j + 1) * CH],
                                pt[:, j * CH:(j + 1) * CH],
                                masks_sb[:, m * CH:(m + 1) * CH],
                                MUL,
                            )
                for j in range(gn):
                    kt = k0 + j
                    nc.tensor.matmul(
                        po[0:65, :],
                        v1[:, kt * 65:(kt + 1) * 65],
                        pt[:, j * CH:(j + 1) * CH],
                        start=(kt == 0), stop=(kt == nk - 1),
                    )
            rec = sb.tile([128, CH], F32, tag="rec", bufs=2,
                          name=f"rec{b}{h}{c}")
            nc.vector.reciprocal(rec[64:65, :], po[64:65, :])
            nc.vector.tensor_copy(
                den_r[64:65, (h * NC_CH + c) * CH:
                      (h * NC_CH + c + 1) * CH],
                rec[64:65, :])
            if (c + h) % 2 == 0:
                nc.vector.tensor_copy(
                    oT[0:64, h * S + qs: h * S + qs + CH], po[0:64, :])
            else:
                nc.scalar.copy(
                    oT[0:64, h * S + qs: h * S + qs + CH], po[0:64, :])
        return run

    def bulk_unit(b, u):
        def run():
            oT, den_r = st_["oT", b], st_["den", b]
            bc = ps.tile([128, 512], F32, tag="bcx", bufs=2, name=f"bc{b}{u}")
            nc.tensor.matmul(
                bc[:, :],
                ones_sb[64:65, 0:128],
                den_r[64:65, u * 512:(u + 1) * 512],
                start=True, stop=True,
            )
            nc.vector.tensor_tensor(
                oT[0:64, u * 512:(u + 1) * 512],
                oT[0:64, u * 512:(u + 1) * 512],
                bc[0:64, :], MUL)
        return run

    def outproj_unit(b, qt, chp):
        def run():
            oT = st_["oT", b]
            r0 = b * S
            pf = ps.tile([128, 512], F32, tag="bcx", bufs=2,
                         name=f"pf{b}{qt}{chp}")
            for half in range(2):
                d0 = chp * 512 + half * CH
                nc.tensor.matmul(
                    pf[:, half * CH:(half + 1) * CH],
                    oT[0:64, qt * 128:(qt + 1) * 128],
                    wo0_sb[:, d0:d0 + CH],
                    start=True, stop=False,
                )
                nc.tensor.matmul(
                    pf[:, half * CH:(half + 1) * CH],
                    oT[0:64, S + qt * 128: S + (qt + 1) * 128],
                    wo1_sb[:, d0:d0 + CH],
                    start=False, stop=True,
                )
            os_ = sb.tile([128, 512], F32, tag="os", bufs=3,
                          name=f"os{b}{qt}{chp}")
            if (qt + chp) % 2 == 0:
                nc.vector.tensor_copy(os_[:, :], pf[:, :])
            else:
                nc.scalar.copy(os_[:, :], pf[:, :])
            nc.sync.dma_start(
                out[r0 + qt * 128: r0 + (qt + 1) * 128,
                    chp * 512:(chp + 1) * 512],
                os_[:, :],
            )
        return run

    def rr(*streams):
        """Round-robin emit: streams = (list_of_units, weight) pairs."""
        idx = [0.0] * len(streams)
        pos = [0] * len(streams)
        while any(pos[i] < len(s) for i, (s, w) in enumerate(streams)):
            for i, (s, w) in enumerate(streams):
                idx[i] += w
                while idx[i] >= 1.0 and pos[i] < len(s):
                    s[pos[i]]()
                    pos[i] += 1
                    idx[i] -= 1.0

    # prologue: QKV(0) + v1(0)
    for cp in range(NC_CH // 2):
        qkv_unit(0, cp)()
    for h in range(2):
        v1_unit(0, h)()

    for b in range(B):
        attn = [attn_unit(b, c, h) for c in range(NC_CH) for h in range(2)]
        if "attn" in ABLATE:
            attn = []
        streams = [(attn, 1.0)]
        if b + 1 < B:
            streams.append(([qkv_unit(b + 1, cp) for cp in range(NC_CH // 2)],
                            4.0 / max(len(attn), 1)))
        if b > 0 and "outproj" not in ABLATE:
            streams.append(([outproj_unit(b - 1, qt, chp)
                             for qt in range(NQT) for chp in range(2)],
                            32.0 / max(len(attn), 1)))
        rr(*streams)
        if b + 1 < B:
            for h in range(2):
                v1_unit(b + 1, h)()
        if "attn" not in ABLATE:
            for u in range(2 * S // 512):
                bulk_unit(b, u)()
    if "outproj" not in ABLATE:
        for qt in range(NQT):
            for chp in range(2):
                outproj_unit(B - 1, qt, chp)()


_NC_CACHE = None


def _get_nc():
    global _NC_CACHE
    if _NC_CACHE is None:
        _NC_CACHE = _build_nc()
    return _NC_CACHE


def kernel(x, Wq, bq, Wk, bk, Wv, bv, Wo, bo):
    global LAST_RESULTS, LAST_IN_MAPS
    x = np.asarray(x, dtype=np.float32)
    Wq = np.asarray(Wq, dtype=np.float32)
    Wk = np.asarray(Wk, dtype=np.float32)
    Wv = np.asarray(Wv, dtype=np.float32)
    Wo = np.asarray(Wo, dtype=np.float32)
    bq = np.asarray(bq, dtype=np.float32)
    bk = np.asarray(bk, dtype=np.float32)
    bv = np.asarray(bv, dtype=np.float32)
    bo = np.asarray(bo, dtype=np.float32)

    xTr = round_fp32r(x.reshape(ROWS, D).T)

    # masks[m][p, f] = 1 if f >= 128*m + p else 0   (m = kt - 2c)
    p = np.arange(128)[:, None]
    f = np.arange(CH)[None, :]
    masks = np.concatenate(
        [(f >= 128 * m + p).astype(np.float32) for m in range(2)], axis=1)
    ident = np.eye(128, dtype=np.float32)
    ones = np.ones((128, 128), dtype=np.float32)

    in_maps = []
    for core in range(NB):
        h0, h1 = 2 * core, 2 * core + 1
        in_maps.append(dict(
            xT=xTr,
            wq=round_fp32r(np.concatenate([Wq[h0], Wq[h1]], axis=1)),
            wk=round_fp32r(np.concatenate([Wk[h0], Wk[h1]], axis=1)),
            wv=round_fp32r(np.concatenate([Wv[h0], Wv[h1]], axis=1)),
            wo0=round_fp32r(Wo[128 * core: 128 * core + 64]),
            wo1=round_fp32r(Wo[128 * core + 64: 128 * core + 128]),
            bq=np.concatenate([bq[h0], bq[h1]])[:, None].astype(np.float32),
            bk=np.concatenate([bk[h0], bk[h1]])[:, None].astype(np.float32),
            bv=np.concatenate([bv[h0], bv[h1]])[:, None].astype(np.float32),
            masks=masks, ident=ident, ones=ones,
        ))

    LAST_IN_MAPS = in_maps
    nc = _get_nc()
    kwargs = {}
    if TRACE:
        kwargs = dict(trace=True, trace_cores=list(range(NB)))
    res = run_bass_kernel_spmd(nc, in_maps, core_ids=list(range(NB)), **kwargs)
    LAST_RESULTS = res

    acc = res.results[0]["out"].astype(np.float32).copy()
    for core in range(1, NB):
        acc += res.results[core]["out"]
    acc += bo[None, :]
    return acc.reshape(B, S, D)



# revision 8
# speedup vs baseline: 1.1422x; 1.1422x over previous
"""Multi-head attention (B=4, S=2048, D=1024, H=16, HS=64, causal) on 8 trn2 cores.

Strategy: tensor-parallel over heads (2 heads per core), x replicated.
Per core: Q/K projections (bf16 matmuls, N=512 moving), V projected
directly into [token, vdim] layout via x-stationary matmuls (no PE
transposes), causal attention with transposed-scores softmax (no
max-subtraction; scores are O(1) by construction), output projection of
the local head pair -> partial [B*S, D] in bf16. Host sums the 8
partials (the Wo contraction over heads).

All matmul operands are bf16 (PSUM accumulation stays fp32); numpy
simulation of this pipeline gives max-rel-err ~4.4e-3 vs the fp32
reference (gate is 2e-2). bf16 halves DMA traffic and allows N=512
moving streams at 1 cycle/row.

Engine budget per core (est): PE ~295us (QK 60, V 42, scores 66, AV 66,
outproj 60), ACT = exp only ~169us, DVE ~190us (bias/v1/rec/norm/os),
GPSIMD ~110us (causal masks, denominator broadcast).
"""

import sys

sys.path.insert(0, "/opt/trn_rl_repo")

import numpy as np
import ml_dtypes

import concourse.bacc as bacc
import concourse.bass as bass
import concourse.mybir as mybir
import concourse.tile as tile
from concourse.bass_utils import run_bass_kernel_spmd

F32 = mybir.dt.float32
F32R = mybir.dt.float32r
BF16 = mybir.dt.bfloat16
EXP = mybir.ActivationFunctionType.Exp
MUL = mybir.AluOpType.mult
ADD = mybir.AluOpType.add
NPBF = ml_dtypes.bfloat16

B, S, D, H, HS = 4, 2048, 1024, 16, 64
ROWS = B * S                      # 8192
NB = 8                            # cores
SCALE = 1.0 / float(np.sqrt(HS))  # 0.125
CH = 256                          # scores/AV moving width
QKCH = 512                       # q/k projection moving width
NC_CH = S // CH                   # 8 q/row chunks per batch
NKT = S // 128                    # 16 k-tiles per batch

TRACE = False
LAST_RESULTS = None
LAST_IN_MAPS = None


def _build_nc(reps: int = 1):
    nc = bacc.Bacc()

    xT = nc.declare_dram_parameter("xT", [D, ROWS], BF16, isOutput=False)
    wq = nc.declare_dram_parameter("wq", [D, 128], BF16, isOutput=False)
    wk = nc.declare_dram_parameter("wk", [D, 128], BF16, isOutput=False)
    wv = nc.declare_dram_parameter("wv", [D, 128], BF16, isOutput=False)
    wo0 = nc.declare_dram_parameter("wo0", [64, D], BF16, isOutput=False)
    wo1 = nc.declare_dram_parameter("wo1", [64, D], BF16, isOutput=False)
    bq = nc.declare_dram_parameter("bq", [128, 1], F32, isOutput=False)
    bk = nc.declare_dram_parameter("bk", [128, 1], F32, isOutput=False)
    bv = nc.declare_dram_parameter("bv", [128, 128], BF16, isOutput=False)
    masks = nc.declare_dram_parameter("masks", [128, 2 * CH], BF16, isOutput=False)
    ones = nc.declare_dram_parameter("ones", [128, 128], F32R, isOutput=False)
    out = nc.declare_dram_parameter("out", [ROWS, D], BF16, isOutput=True)

    with tile.TileContext(nc) as tc:
        with tc.tile_pool(name="const", bufs=1) as cpool, \
             tc.tile_pool(name="sb", bufs=1) as sb, \
             tc.tile_pool(name="ps", bufs=1, space="PSUM") as ps:

            # ---- persistent constants ----
            wq_sb = cpool.tile([128, 8 * 128], BF16)
            wk_sb = cpool.tile([128, 8 * 128], BF16)
            wv_sb = cpool.tile([128, 8 * 128], BF16)
            for w_sb, w in ((wq_sb, wq), (wk_sb, wk), (wv_sb, wv)):
                nc.sync.dma_start(
                    w_sb[:, :].rearrange("p (dk j) -> p dk j", dk=8),
                    w[:, :].rearrange("(dk p) j -> p dk j", dk=8),
                )
            wo0_sb = cpool.tile([64, D], BF16)
            wo1_sb = cpool.tile([64, D], BF16)
            nc.sync.dma_start(wo0_sb[:, :], wo0[:, :])
            nc.sync.dma_start(wo1_sb[:, :], wo1[:, :])
            bq_sb = cpool.tile([128, 1], F32)
            bk_sb = cpool.tile([128, 1], F32)
            bv_sb = cpool.tile([128, 128], BF16)
            nc.sync.dma_start(bq_sb[:, :], bq[:, :])
            nc.sync.dma_start(bk_sb[:, :], bk[:, :])
            nc.sync.dma_start(bv_sb[:, :], bv[:, :])
            masks_sb = cpool.tile([128, 2 * CH], BF16)
            nc.sync.dma_start(masks_sb[:, :], masks[:, :])
            ones_sb = cpool.tile([128, 128], F32R)
            nc.sync.dma_start(ones_sb[:, :], ones[:, :])

            env = dict(
                wq_sb=wq_sb, wk_sb=wk_sb, wv_sb=wv_sb, wo0_sb=wo0_sb,
                wo1_sb=wo1_sb, bq_sb=bq_sb, bk_sb=bk_sb, bv_sb=bv_sb,
                masks_sb=masks_sb, ones_sb=ones_sb, xT=xT, out=out)

            if reps > 1:
                with tc.For_i(0, reps, 1):
                    _emit(nc, tc, sb, ps, env)
            else:
                _emit(nc, tc, sb, ps, env)
    nc.compile()
    return nc


def _emit(nc, tc, sb, ps, env):
    wq_sb = env["wq_sb"]; wk_sb = env["wk_sb"]; wv_sb = env["wv_sb"]
    wo0_sb = env["wo0_sb"]; wo1_sb = env["wo1_sb"]
    bq_sb = env["bq_sb"]; bk_sb = env["bk_sb"]; bv_sb = env["bv_sb"]
    masks_sb = env["masks_sb"]; ones_sb = env["ones_sb"]
    xT = env["xT"]; out = env["out"]

    st_ = {}   # per-b tiles

    def qkv_unit(b, cp):
        def run():
            r0 = b * S
            if cp == 0:
                st_["qT", b] = sb.tile([128, S], BF16, tag="qT", bufs=2,
                                       name=f"qT{b}")
                st_["kT", b] = sb.tile([128, S], BF16, tag="kT", bufs=2,
                                       name=f"kT{b}")
                for h in range(2):
                    v1 = sb.tile([128, NKT * 65], BF16, tag="v1", bufs=4,
                                 name=f"v1_{b}_{h}")
                    st_["v1", b, h] = v1
                    ones_dst = bass.AP(
                        v1.tensor, v1.offset + 64,
                        [v1.ap[0]] + [[65, NKT]])
                    nc.vector.tensor_copy(ones_dst, ones_sb[:, 0:NKT])
            qT, kT = st_["qT", b], st_["kT", b]
            xt = sb.tile([128, 8 * QKCH], BF16, tag="xt", bufs=2,
                         name=f"xt{b}_{cp}")
            nc.sync.dma_start(
                xt[:, :].rearrange("p (dk j) -> p dk j", dk=8),
                xT[:, r0 + cp * QKCH: r0 + (cp + 1) * QKCH]
                .rearrange("(dk p) j -> p dk j", dk=8),
            )
            for w_sb, dest, bias in ((wq_sb, qT, bq_sb), (wk_sb, kT, bk_sb)):
                pp = ps.tile([128, QKCH], F32, tag="st", bufs=2,
                             name=f"pp{b}{cp}")
                for dk in range(8):
                    nc.tensor.matmul(
                        pp[:, :],
                        w_sb[:, dk * 128:(dk + 1) * 128],
                        xt[:, dk * QKCH:(dk + 1) * QKCH],
                        start=(dk == 0), stop=(dk == 7),
                    )
                nc.vector.tensor_scalar_add(
                    dest[:, cp * QKCH:(cp + 1) * QKCH], pp[:, :], bias[:, :])
            # V directly in [token, vdim] layout: x chunk stationary,
            # Wv moving (both heads at once).
            for qt in range(QKCH // 128):
                pv = ps.tile([128, 128], F32, tag="bcx", bufs=2,
                             name=f"pv{b}{cp}{qt}")
                for dk in range(8):
                    nc.tensor.matmul(
                        pv[:, :],
                        xt[:, dk * QKCH + qt * 128: dk * QKCH + (qt + 1) * 128],
                        wv_sb[:, dk * 128:(dk + 1) * 128],
                        start=(dk == 0), stop=(dk == 7),
                    )
                kt = cp * (QKCH // 128) + qt
                for h in range(2):
                    v1 = st_["v1", b, h]
                    nc.vector.tensor_tensor(
                        v1[:, kt * 65: kt * 65 + 64],
                        pv[:, h * 64:(h + 1) * 64],
                        bv_sb[:, h * 64:(h + 1) * 64],
                        ADD,
                    )
        return run

    def attn_unit(b, c, h):
        def run():
            if ("oT", b) not in st_:
                st_["oT", b] = sb.tile([64, 2 * S], BF16, tag="oT", bufs=2,
                                       name=f"oT{b}")
            oT = st_["oT", b]
            qT, kT = st_["qT", b], st_["kT", b]
            v1 = st_["v1", b, h]
            hp = h * 64
            qs = c * CH
            nk = 2 * (c + 1)
            po = ps.tile([128, CH], F32, tag="po", bufs=2, name=f"po{b}{h}{c}")
            for g in range((nk + 3) // 4):
                k0 = 4 * g
                gn = min(4, nk - k0)
                stp = ps.tile([128, 1024], F32, tag="st", bufs=2,
                              name=f"stp{b}{h}{c}{g}")
                for j in range(gn):
                    nc.tensor.matmul(
                        stp[:, j * CH:(j + 1) * CH],
                        kT[hp:hp + 64, (k0 + j) * 128:(k0 + j + 1) * 128],
                        qT[hp:hp + 64, qs:qs + CH],
                        start=True, stop=True,
                    )
                pt = sb.tile([128, 1024], BF16, tag="pt", bufs=4,
                             name=f"pt{b}{h}{c}{g}")
                nc.scalar.activation(pt[:, 0:gn * CH], stp[:, 0:gn * CH],
                                     EXP, scale=SCALE)
                for j in range(gn):
                    m = (k0 + j) - 2 * c
                    if m >= 0:
                        nc.vector.tensor_tensor(
                            pt[:, j * CH:(j + 1) * CH],
                            pt[:, j * CH:(j + 1) * CH],
                            masks_sb[:, m * CH:(m + 1) * CH],
                            MUL,
                        )
                for j in range(gn):
                    kt = k0 + j
                    nc.tensor.matmul(
                        po[0:65, :],
                        v1[:, kt * 65:(kt + 1) * 65],
                        pt[:, j * CH:(j + 1) * CH],
                        start=(kt == 0), stop=(kt == nk - 1),
                    )
            rec = sb.tile([128, CH], F32R, tag="rec", bufs=2,
                          name=f"rec{b}{h}{c}")
            with nc.allow_low_precision(reason="f32r reciprocal (12-bit) ok"):
                nc.vector.reciprocal(rec[64:65, :], po[64:65, :])
            bcp = ps.tile([128, CH], F32, tag="bcx", bufs=2,
                          name=f"bcp{b}{h}{c}")
            nc.tensor.matmul(
                bcp[:, :], ones_sb[64:65, 0:128], rec[64:65, :],
                start=True, stop=True,
            )
            bc = sb.tile([128, CH], F32, tag="bc", bufs=2,
                         name=f"bc{b}{h}{c}")
            if (c + h) % 2 == 0:
                nc.vector.tensor_copy(bc[:, :], bcp[:, :])
            else:
                nc.scalar.copy(bc[:, :], bcp[:, :])
            nc.vector.tensor_tensor(
                oT[0:64, h * S + qs: h * S + qs + CH],
                po[0:64, :], bc[0:64, :], MUL)
        return run

    def outproj_unit(b, qt, chp):
        def run():
            oT = st_["oT", b]
            r0 = b * S
            pf = ps.tile([128, 512], F32, tag="bcx", bufs=2,
                         name=f"pf{b}{qt}{chp}")
            d0 = chp * 512
            nc.tensor.matmul(
                pf[:, :],
                oT[0:64, qt * 128:(qt + 1) * 128],
                wo0_sb[:, d0:d0 + 512],
                start=True, stop=False,
            )
            nc.tensor.matmul(
                pf[:, :],
                oT[0:64, S + qt * 128: S + (qt + 1) * 128],
                wo1_sb[:, d0:d0 + 512],
                start=False, stop=True,
            )
            os_ = sb.tile([128, 512], BF16, tag="os", bufs=3,
                          name=f"os{b}{qt}{chp}")
            if (qt + chp) % 2 == 0:
                nc.vector.tensor_copy(os_[:, :], pf[:, :])
            else:
                nc.scalar.copy(os_[:, :], pf[:, :])
            nc.sync.dma_start(
                out[r0 + qt * 128: r0 + (qt + 1) * 128,
                    chp * 512:(chp + 1) * 512],
                os_[:, :],
            )
        return run

    def rr(*streams):
        """Round-robin emit: streams = (list_of_units, weight) pairs."""
        idx = [0.0] * len(streams)
        pos = [0] * len(streams)
        while any(pos[i] < len(s) for i, (s, w) in enumerate(streams)):
            for i, (s, w) in enumerate(streams):
                idx[i] += w
                while idx[i] >= 1.0 and pos[i] < len(s):
                    s[pos[i]]()
                    pos[i] += 1
                    idx[i] -= 1.0

    # prologue: QKV(0)
    for cp in range(S // QKCH):
        qkv_unit(0, cp)()

    for b in range(B):
        # attention for batch b with batch b's outproj interleaved at a
        # one-chunk lag (outproj qt needs attn chunk qt//2 of both heads).
        attn = []
        for c in range(NC_CH):
            for h in range(2):
                attn.append(attn_unit(b, c, h))
            if c >= 1:
                for qt in (2 * (c - 1), 2 * (c - 1) + 1):
                    for chp in range(2):
                        attn.append(outproj_unit(b, qt, chp))
        streams = [(attn, 1.0)]
        if b + 1 < B:
            streams.append(([qkv_unit(b + 1, cp) for cp in range(S // QKCH)],
                            (S // QKCH) / len(attn)))
        rr(*streams)
        for qt in (2 * (NC_CH - 1), 2 * (NC_CH - 1) + 1):
            for chp in range(2):
                outproj_unit(b, qt, chp)()


_NC_CACHE = None


def _get_nc():
    global _NC_CACHE
    if _NC_CACHE is None:
        _NC_CACHE = _build_nc()
    return _NC_CACHE


def kernel(x, Wq, bq, Wk, bk, Wv, bv, Wo, bo):
    global LAST_RESULTS, LAST_IN_MAPS
    x = np.asarray(x, dtype=np.float32)
    Wq = np.asarray(Wq, dtype=np.float32)
    Wk = np.asarray(Wk, dtype=np.float32)
    Wv = np.asarray(Wv, dtype=np.float32)
    Wo = np.asarray(Wo, dtype=np.float32)
    bq = np.asarray(bq, dtype=np.float32)
    bk = np.asarray(bk, dtype=np.float32)
    bv = np.asarray(bv, dtype=np.float32)
    bo = np.asarray(bo, dtype=np.float32)

    xTb = np.ascontiguousarray(x.reshape(ROWS, D).T).astype(NPBF)

    # masks[m][p, f] = 1 if f >= 128*m + p else 0   (m = kt - 2c)
    p = np.arange(128)[:, None]
    f = np.arange(CH)[None, :]
    masks = np.concatenate(
        [(f >= 128 * m + p).astype(NPBF) for m in range(2)], axis=1)
    ones = np.ones((128, 128), dtype=np.float32)

    in_maps = []
    for core in range(NB):
        h0, h1 = 2 * core, 2 * core + 1
        bv_cat = np.concatenate([bv[h0], bv[h1]])            # [128]
        in_maps.append(dict(
            xT=xTb,
            wq=np.concatenate([Wq[h0], Wq[h1]], axis=1).astype(NPBF),
            wk=np.concatenate([Wk[h0], Wk[h1]], axis=1).astype(NPBF),
            wv=np.concatenate([Wv[h0], Wv[h1]], axis=1).astype(NPBF),
            wo0=Wo[128 * core: 128 * core + 64].astype(NPBF),
            wo1=Wo[128 * core + 64: 128 * core + 128].astype(NPBF),
            bq=np.concatenate([bq[h0], bq[h1]])[:, None].astype(np.float32),
            bk=np.concatenate([bk[h0], bk[h1]])[:, None].astype(np.float32),
            bv=np.broadcast_to(bv_cat[None, :], (128, 128)).astype(NPBF),
            masks=masks, ones=ones,
        ))

    LAST_IN_MAPS = in_maps
    nc = _get_nc()
    kwargs = {}
    if TRACE:
        kwargs = dict(trace=True, trace_cores=list(range(NB)))
    res = run_bass_kernel_spmd(nc, in_maps, core_ids=list(range(NB)), **kwargs)
    LAST_RESULTS = res

    acc = res.results[0]["out"].astype(np.float32)
    for core in range(1, NB):
        acc = acc + res.results[core]["out"].astype(np.float32)
    acc += bo[None, :]
    return acc.reshape(B, S, D)
